# revision 90
# baseline (speedup 1.0000x reference)
"""Bass/Trainium2 kernel for the Show-Attend-Tell style attention module.

Reference math (per batch b):
    enc_att = encoder_out @ W_enc + b_enc            # [L, A]
    dec_att = decoder_hidden @ W_dec + b_dec         # [A]
    hidden  = relu(enc_att + dec_att)                # [L, A]
    att     = hidden @ W_att (+ b_att)               # [L]   (b_att cancels in softmax)
    alpha   = softmax(att)                           # [L]
    awe     = alpha @ encoder_out                    # [E]

Distribution: pure data-parallel over the batch dim, 32 batches per core,
no collectives.  The host shard step supplies the encoder shard in both
layouts the PE needs (token-major for the weighted sum, feature-major for
the projection), pre-cast to bf16 so the total HBM bytes equal one fp32
read of the tensor.

On-chip layout is "transposed" (feature-on-partitions):
    encT  [128(e), 16, L]   per batch  -> moving operand of the projection
    hidT  [128(a), 4, L]    per batch  -> moving operand of the att matvec
    att rows are gathered to [8, L], softmax'd batched, then alpha is
    PE-transposed and scattered into a block-diagonal [13*128, 8] stationary
    operand so the weighted sum runs as one accumulated matmul over the
    group's flattened (b, l) rows.
"""

import functools
import os

import ml_dtypes
import numpy as np

import concourse.bacc as bacc
import concourse.mybir as mybir
import concourse.tile as tile
from concourse.bass_utils import run_bass_kernel_spmd

N_CORES = 8
B, L, ENC, DEC, ATT = 256, 196, 2048, 512, 512
BC = B // N_CORES          # 32 batches per core
GROUP = 8                  # batches per awe-group
N_GROUPS = BC // GROUP     # 8
NPAIR = 2                  # batches processed per PE pass
# the weighted sum packs each batch's L=196 rows into a 224-row band
# (224 = 7*32) so a group is exactly 7 full 128-row chunks and every
# alpha^T fragment starts at a 32-aligned partition (DVE requirement)
PADL = 224
K_CHUNKS = GROUP * PADL // 128  # 7
OUT_W = ENC + L            # packed output row: awe | alpha


def _pieces(span):
    """Per group-local batch: (chunk, part, l0, n) pieces covering l in
    [0, span), split at 128-row chunk boundaries of the padded layout."""
    out = []
    for jj in range(GROUP):
        row0 = jj * PADL
        ps = []
        s = row0
        while s < row0 + span:
            e = min(row0 + span, (s // 128 + 1) * 128)
            ps.append((s // 128, s % 128, s - row0, e - s))
            s = e
        out.append(ps)
    return out


def _split_for_engine(pieces):
    """DVE/ACT partition windows: start 0 -> <=128, start 64 -> <=64,
    start 32/96 -> <=32. Split fragments to fit."""
    out = []
    for ps in pieces:
        cur = []
        for chunk, part, l0, n in ps:
            while n > 0:
                lim = 128 if part == 0 else 64 if part == 64 else 32
                take = min(n, lim)
                cur.append((chunk, part, l0, take))
                part += take
                l0 += take
                n -= take
        out.append(cur)
    return out


PIECES_VAL = _split_for_engine(_pieces(L))  # alpha fragments (32-aligned)
PIECES_PAD = _pieces(PADL)  # nat DMA coverage incl. pad rows
assert all(p % 32 == 0 for ps in PIECES_VAL for _, p, _, _ in ps)

F32 = mybir.dt.float32
BF16 = mybir.dt.bfloat16
FP8 = mybir.dt.float8e4
AX = mybir.AxisListType.X
AF = mybir.ActivationFunctionType

# fp8 (e4m3, DoubleRow) path for the big projection matmul: 2x PE rate.
# Whole-chain rel err ~1e-2 (vs 2.5e-3 bf16), still under the 2e-2 gate.
USE_FP8 = True
SWI = True                 # DoubleRowSwInterleave: host-interleaved weights
W_SCALE = 4096.0           # W_enc pre-scale so e4m3 sees normal-range values
LP = 208                   # L padded to a multiple of 16 for the DoubleRow AP

TRACE_ENV = "ATT_KERNEL_TRACE"
LAST_RESULTS = {}


def build_nc():
    nc = bacc.Bacc(
        "TRN2", target_bir_lowering=False, debug=False, num_devices=N_CORES
    )
    # enc_t host layout (bf16): [b][p][c][l] = enc[b, l, 128*c + p] so each
    # DMA is contiguous per partition (6272 B runs, not 392 B descriptors).
    # fp8 DoubleRow layout: [b][p][c8][r][l] = enc[b, l, 256*c8 + 2*p + r].
    if USE_FP8:
        # pair-interleaved: [pair][p][c8][r][jj][l] = enc[2*pair+jj, l, 256*c8+2*p+r]
        enc_t = nc.declare_dram_parameter(
            "enc_t", [BC // NPAIR, 128, ENC // 256, 2, NPAIR, LP], FP8, isOutput=False
        )
    else:
        enc_t = nc.declare_dram_parameter(
            "enc_t", [BC, 128, (ENC // 128) * L], BF16, isOutput=False
        )
    enc_nat = nc.declare_dram_parameter("enc_nat", [BC * L, ENC], BF16, isOutput=False)
    # packed small preloads, one DMA each:
    # f32 blob [128, 40]: b_enc [*,0:4] | b_dec [*,4:8] | ident rows [*,8:40]
    small_f32 = nc.declare_dram_parameter("small_f32", [128, 40], F32, isOutput=False)
    # bf16 blob [128, 132]: dec_h^T [*, 0:128] ([p][q][b] = dec[b, 128q+p]) |
    # w_att [*, 128:132]
    small_bf = nc.declare_dram_parameter("small_bf", [128, 132], BF16, isOutput=False)
    if USE_FP8:
        # m-major so the first m-slice (256 KB) lands fast at kernel start
        w_enc = nc.declare_dram_parameter(
            "w_enc", [ATT // 128, 128, ENC // 256, 256], FP8, isOutput=False
        )
    else:
        w_enc = nc.declare_dram_parameter("w_enc", [ENC, ATT], BF16, isOutput=False)
    w_dec = nc.declare_dram_parameter("w_dec", [DEC, ATT], BF16, isOutput=False)
    out_d = nc.declare_dram_parameter("out", [BC, OUT_W], F32, isOutput=True)

    with tile.TileContext(nc) as tc:
        with (
            tc.tile_pool(name="w", bufs=1) as wp,
            tc.tile_pool(name="encT", bufs=2) as encT_p,
            tc.tile_pool(name="nat", bufs=2) as nat_p,
            tc.tile_pool(name="hid", bufs=3) as hid_p,
            tc.tile_pool(name="bd", bufs=3) as bd_p,
            tc.tile_pool(name="sm", bufs=4) as sm_p,
            tc.tile_pool(name="awe", bufs=2) as awe_p,
            tc.tile_pool(name="psmm", bufs=3, space="PSUM") as ps_mm,
            tc.tile_pool(name="psatt", bufs=1, space="PSUM") as ps_att,
            tc.tile_pool(name="psawe", bufs=2, space="PSUM") as ps_awe,
            tc.tile_pool(name="pssm", bufs=1, space="PSUM") as ps_sm,
        ):
            # ---- constant / weight preloads: two packed small DMAs on the
            # gpsimd ring, w_enc m-slices first on sync so pair 0 starts fast
            sf = wp.tile([128, 40], F32)
            nc.gpsimd.dma_start(sf[:], small_f32.ap())
            sb = wp.tile([128, 132], BF16)
            nc.gpsimd.dma_start(sb[:], small_bf.ap())
            b_enc_sb = sf[:, 0:4]
            b_dec_sb = sf[:, 4:8]
            ident = sf[0:32, 8:40]
            dect_sb = sb[:, 0:128].rearrange("p (q b) -> p q b", b=BC)
            w_att_sb = sb[:, 128:132]
            if USE_FP8:
                w_enc_sb = wp.tile([128, ATT // 128, ENC // 256, 256], FP8)
                for m in range(ATT // 128):
                    nc.sync.dma_start(w_enc_sb[:, m], w_enc.ap()[m])
            else:
                w_enc_sb = wp.tile([128, ENC // 128, ATT], BF16)
                nc.sync.dma_start(
                    w_enc_sb[:], w_enc.ap().rearrange("(c p) a -> p c a", p=128)
                )
            w_dec_sb = wp.tile([128, DEC // 128, ATT], BF16)
            nc.sync.dma_start(
                w_dec_sb[:], w_dec.ap().rearrange("(c p) a -> p c a", p=128)
            )
            bias_sb = wp.tile([128, ATT // 128], F32)
            nc.vector.tensor_add(bias_sb[:], b_enc_sb, b_dec_sb)

            # dec_att2^T = W_dec^T @ dec_hidden^T + (b_enc + b_dec), f32: the
            # per-partition relu bias. Emitted late (inside pair 0, after its
            # first matmul block) so it does not gate the kernel start.
            dec2_sb = wp.tile([128, ATT // 128, BC], F32)

            def emit_dec_chain():
                for m in range(ATT // 128):
                    pm = ps_sm.tile([128, BC], F32, tag="sm", name=f"decmm{m}")
                    for k in range(DEC // 128):
                        nc.tensor.matmul(
                            pm[:],
                            w_dec_sb[:, k, m * 128 : (m + 1) * 128],
                            dect_sb[:, k, :],
                            start=(k == 0),
                            stop=(k == DEC // 128 - 1),
                        )
                    nc.vector.tensor_scalar_add(
                        dec2_sb[:, m, :], pm[:], bias_sb[:, m : m + 1]
                    )

            # nat rows for batch jj of group g: pad rows deliberately read the
            # following batch's rows (they are multiplied by bd zeros); the
            # very last batch clamps to re-read valid trailing rows instead
            def emit_nat_loads(g, jj, nat):
                b = g * GROUP + jj
                for chunk, part, l0, n in PIECES_PAD[jj]:
                    src0 = b * L + l0
                    if src0 + n <= BC * L:
                        nc.sync.dma_start(
                            nat[part : part + n, chunk, :],
                            enc_nat.ap()[src0 : src0 + n, :],
                        )
                    else:
                        nv = BC * L - src0
                        if nv > 0:
                            nc.sync.dma_start(
                                nat[part : part + nv, chunk, :],
                                enc_nat.ap()[src0 : src0 + nv, :],
                            )
                        npad = n - nv
                        nc.sync.dma_start(
                            nat[part + nv : part + n, chunk, :],
                            enc_nat.ap()[BC * L - npad : BC * L, :],
                        )

            # awe for group g, emitted one group late so the PE never waits
            # on the softmax -> alpha^T -> bd chain
            def do_awe(g, nat, bd):
                awe_sb = awe_p.tile([GROUP, ENC], F32, tag="awesb")
                for n in range(ENC // 512):
                    pw = ps_awe.tile([GROUP, 512], F32, tag="pw", name=f"pw_{g}_{n}")
                    for c in range(K_CHUNKS):
                        nc.tensor.matmul(
                            pw[:],
                            bd[:, c, :],
                            nat[:, c, n * 512 : (n + 1) * 512],
                            start=(c == 0),
                            stop=(c == K_CHUNKS - 1),
                        )
                    nc.vector.tensor_copy(awe_sb[:, n * 512 : (n + 1) * 512], pw[:])
                nc.sync.dma_start(
                    out_d.ap()[g * GROUP : (g + 1) * GROUP, 0:ENC], awe_sb[:]
                )

            # ---- main loop ------------------------------------------------
            pending = None
            dec_emitted = [False]

            def maybe_emit_dec():
                if not dec_emitted[0]:
                    dec_emitted[0] = True
                    emit_dec_chain()

            for g in range(N_GROUPS):
                # group's encoder rows, 128-padded per batch: chunk 2j holds
                # batch j's rows l=0:128, chunk 2j+1 rows l=128:196 (68 valid)
                # group's encoder rows for the weighted sum (consumed by
                # do_awe(g) one iteration later); sync ring so they never
                # delay the critical encT stream on gpsimd, interleaved into
                # the pair loop to smooth the queue
                nat = nat_p.tile([128, K_CHUNKS, ENC], BF16)

                bd = bd_p.tile([128, K_CHUNKS, GROUP], BF16)
                nc.vector.memset(bd[:], 0.0)

                for t in range(GROUP // NPAIR):
                    b0 = g * GROUP + NPAIR * t
                    hid = hid_p.tile([128, ATT // 128, NPAIR, L], BF16)
                    if USE_FP8:
                        encT = encT_p.tile(
                            [128, ENC // 256, 2, NPAIR, LP], FP8, tag="encT"
                        )
                        if g == 0 and t == 0:
                            # split the very first load so the m-loop can
                            # start after half the pair's encoder lands
                            hc = ENC // 512
                            nc.gpsimd.dma_start(
                                encT[:, 0:hc], enc_t.ap()[b0 // NPAIR, :, 0:hc]
                            )
                            nc.gpsimd.dma_start(
                                encT[:, hc:], enc_t.ap()[b0 // NPAIR, :, hc:]
                            )
                        else:
                            nc.gpsimd.dma_start(encT[:], enc_t.ap()[b0 // NPAIR])
                        # group 0: keep HBM clear for the critical first encT
                        # transfers; its nat loads are emitted after the pairs
                        if g > 0:
                            for jj in range(NPAIR * t, NPAIR * (t + 1)):
                                emit_nat_loads(g, jj, nat)
                        for m in range(ATT // 128):
                            pm = ps_mm.tile([128, NPAIR, LP], F32, tag="pm")
                            for c in range(ENC // 256):
                                if SWI:
                                    lw = w_enc_sb[:, m, c, :]
                                    pmode = mybir.MatmulPerfMode.DoubleRowSwInterleave
                                else:
                                    lw = w_enc_sb[:, m, c, :].rearrange(
                                        "p (r mm) -> p r mm", r=2
                                    )
                                    pmode = mybir.MatmulPerfMode.DoubleRow
                                nc.tensor.matmul(
                                    pm[:],
                                    lw,
                                    encT[:, c, :, :, :].rearrange(
                                        "p r j l -> p r (j l)"
                                    ),
                                    start=(c == 0),
                                    stop=(c == ENC // 256 - 1),
                                    perf_mode=pmode,
                                )
                            maybe_emit_dec()
                            for j in range(NPAIR):
                                nc.scalar.activation(
                                    hid[:, m, j, :],
                                    pm[:, j, 0:L],
                                    AF.Relu,
                                    bias=dec2_sb[:, m, b0 + j : b0 + j + 1],
                                    scale=1.0 / W_SCALE,
                                )
                    else:
                        encT = encT_p.tile(
                            [128, NPAIR, ENC // 128, L], BF16, tag="encT"
                        )
                        for j in range(NPAIR):
                            nc.gpsimd.dma_start(
                                encT[:, j, :, :],
                                enc_t.ap()[b0 + j].rearrange("p (c l) -> p c l", l=L),
                            )
                        for jj in range(NPAIR * t, NPAIR * (t + 1)):
                            b = g * GROUP + jj
                            nc.sync.dma_start(
                                nat[:, 2 * jj, :], enc_nat.ap()[b * L : b * L + L0, :]
                            )
                            nc.sync.dma_start(
                                nat[0:L1, 2 * jj + 1, :],
                                enc_nat.ap()[b * L + L0 : (b + 1) * L, :],
                            )
                        for m in range(ATT // 128):
                            pm = ps_mm.tile([128, NPAIR, L], F32, tag="pm")
                            for c in range(ENC // 128):
                                nc.tensor.matmul(
                                    pm[:],
                                    w_enc_sb[:, c, m * 128 : (m + 1) * 128],
                                    encT[:, :, c, :],
                                    start=(c == 0),
                                    stop=(c == ENC // 128 - 1),
                                )
                            maybe_emit_dec()
                            for j in range(NPAIR):
                                nc.scalar.activation(
                                    hid[:, m, j, :],
                                    pm[:, j, :],
                                    AF.Relu,
                                    bias=dec2_sb[:, m, b0 + j : b0 + j + 1],
                                    scale=1.0,
                                )
                    pa = ps_att.tile([1, NPAIR, L], F32)
                    for m in range(ATT // 128):
                        nc.tensor.matmul(
                            pa[:],
                            w_att_sb[:, m : m + 1],
                            hid[:, m, :, :],
                            start=(m == 0),
                            stop=(m == ATT // 128 - 1),
                        )

                    # softmax in row form (all ops live on partition 0)
                    negmax = sm_p.tile([1, NPAIR, 1], F32, tag="negmax")
                    nc.vector.reduce_max(negmax[:], pa[:], axis=AX, negate=True)
                    p_pr = sm_p.tile([1, NPAIR, L], F32, tag="ppr")
                    for j in range(NPAIR):
                        nc.scalar.activation(
                            p_pr[:, j, :], pa[:, j, :], AF.Exp,
                            bias=negmax[:, j, :], scale=1.0,
                        )
                    ssum = sm_p.tile([1, NPAIR, 1], F32, tag="ssum")
                    nc.vector.reduce_sum(ssum[:], p_pr[:], axis=AX)
                    rcp = sm_p.tile([1, NPAIR, 1], F32, tag="rcp")
                    nc.vector.reciprocal(rcp[:], ssum[:])
                    alpha_pr = sm_p.tile([1, NPAIR, L], F32, tag="alphapr")
                    for j in range(NPAIR):
                        nc.vector.tensor_scalar_mul(
                            alpha_pr[:, j, :], p_pr[:, j, :], rcp[:, j, :]
                        )
                    nc.sync.dma_start(
                        out_d.ap()[b0 : b0 + NPAIR, ENC : ENC + L], alpha_pr[:]
                    )
                    # alpha^T fragments into the block-diagonal stationary
                    # operand; every fragment starts 32-aligned by layout
                    for j in range(NPAIR):
                        jj = NPAIR * t + j
                        for idx, (chunk, part, l0, n) in enumerate(PIECES_VAL[jj]):
                            ta = ps_sm.tile(
                                [n, 1], F32, tag="at", name=f"at{g}_{jj}_{idx}"
                            )
                            nc.tensor.transpose(
                                ta[:], alpha_pr[:, j, l0 : l0 + n], ident[0:1, 0:1]
                            )
                            nc.vector.tensor_copy(
                                bd[part : part + n, chunk, jj : jj + 1], ta[:]
                            )

                if g == 0:
                    for jj in range(GROUP):
                        emit_nat_loads(g, jj, nat)
                if pending is not None:
                    do_awe(*pending)
                pending = (g, nat, bd)
            do_awe(*pending)

    nc.compile()
    return nc


@functools.cache
def _get_nc():
    return build_nc()


def make_in_maps(encoder_out, decoder_hidden, W_enc, b_enc, W_dec, b_dec, W_att):
    bf = ml_dtypes.bfloat16
    f8 = ml_dtypes.float8_e4m3
    enc = np.ascontiguousarray(np.asarray(encoder_out, dtype=np.float32))
    dec = np.ascontiguousarray(np.asarray(decoder_hidden, dtype=np.float32))
    if USE_FP8:
        # [m][p][c8][r][a'] = (W_SCALE * W)[256*c8 + 2*p + r, 128*m + a']
        w5 = (
            (np.asarray(W_enc, np.float32) * W_SCALE)
            .reshape(ENC // 256, 128, 2, ATT // 128, 128)
            .transpose(3, 1, 0, 2, 4)
        )
        if SWI:
            # hw flat layout: q = 2*(127 - a') + r  (A/B interleaved per
            # column, columns reversed — see CoreSim DoubleRowSwInterleave)
            w5 = w5[:, :, :, :, ::-1].transpose(0, 1, 2, 4, 3)
        w_enc_bf = np.ascontiguousarray(w5).astype(f8).reshape(
            ATT // 128, 128, ENC // 256, 256
        )
    else:
        w_enc_bf = np.ascontiguousarray(np.asarray(W_enc)).astype(bf)
    w_dec_bf = np.ascontiguousarray(np.asarray(W_dec)).astype(bf)
    w_att_bf = np.ascontiguousarray(np.asarray(W_att)).astype(bf)
    b_enc32 = np.ascontiguousarray(np.asarray(b_enc, dtype=np.float32))
    b_dec32 = np.ascontiguousarray(np.asarray(b_dec, dtype=np.float32))
    ident = np.eye(32, dtype=np.float32)
    in_maps = []
    for i in range(N_CORES):
        sh = enc[i * BC : (i + 1) * BC]
        sh_bf = sh.astype(bf)
        if USE_FP8:
            # [pair][p][c8][r][jj][l] = enc[2*pair+jj, l, 256*c8 + 2*p + r]
            et = np.zeros((BC // NPAIR, 128, ENC // 256, 2, NPAIR, LP), dtype=f8)
            et[..., :L] = (
                sh.astype(f8)
                .transpose(0, 2, 1)
                .reshape(BC // NPAIR, NPAIR, ENC // 256, 128, 2, L)
                .transpose(0, 3, 2, 4, 1, 5)
            )
            enc_t = et
        else:
            enc_t = np.ascontiguousarray(
                sh_bf.transpose(0, 2, 1)
                .reshape(BC, ENC // 128, 128, L)
                .transpose(0, 2, 1, 3)
            ).reshape(BC, 128, (ENC // 128) * L)
        in_maps.append(
            {
                "enc_t": enc_t,
                "enc_nat": sh_bf.reshape(BC * L, ENC),
                "small_f32": np.concatenate(
                    [
                        b_enc32.reshape(ATT // 128, 128).T,
                        b_dec32.reshape(ATT // 128, 128).T,
                        np.concatenate(
                            [ident, np.zeros((96, 32), np.float32)], axis=0
                        ),
                    ],
                    axis=1,
                ).astype(np.float32),
                "small_bf": np.concatenate(
                    [
                        np.ascontiguousarray(
                            dec[i * BC : (i + 1) * BC]
                            .T.reshape(DEC // 128, 128, BC)
                            .transpose(1, 0, 2)
                        ).reshape(128, 128),
                        w_att_bf.reshape(ATT // 128, 128).T,
                    ],
                    axis=1,
                ).astype(bf),
                "w_enc": w_enc_bf,
                "w_dec": w_dec_bf,
            }
        )
    return in_maps


def kernel(
    encoder_out, decoder_hidden, W_enc, b_enc, W_dec, b_dec, W_att, b_att=None
):
    nc = _get_nc()
    in_maps = make_in_maps(
        encoder_out, decoder_hidden, W_enc, b_enc, W_dec, b_dec, W_att
    )
    trace = bool(int(os.environ.get(TRACE_ENV, "0")))
    kw = {}
    if trace:
        kw["trace"] = True
    res = run_bass_kernel_spmd(nc, in_maps, core_ids=list(range(N_CORES)), **kw)
    LAST_RESULTS["exec_time_ns"] = res.exec_time_ns
    LAST_RESULTS["mean_exec_time_ns"] = res.mean_exec_time_ns
    full = np.concatenate([r["out"] for r in res.results], axis=0)
    awe = np.ascontiguousarray(full[:, :ENC])
    alpha = np.ascontiguousarray(full[:, ENC:])
    return awe, alpha


# revision 91
# speedup vs baseline: 1.0561x; 1.0561x over previous
"""Bass/Trainium2 kernel for the Show-Attend-Tell style attention module.

Reference math (per batch b):
    enc_att = encoder_out @ W_enc + b_enc            # [L, A]
    dec_att = decoder_hidden @ W_dec + b_dec         # [A]
    hidden  = relu(enc_att + dec_att)                # [L, A]
    att     = hidden @ W_att (+ b_att)               # [L]   (b_att cancels in softmax)
    alpha   = softmax(att)                           # [L]
    awe     = alpha @ encoder_out                    # [E]

Distribution: pure data-parallel over the batch dim, 32 batches per core,
no collectives.  The host shard step supplies the encoder shard in both
layouts the PE needs (token-major for the weighted sum, feature-major for
the projection), pre-cast to bf16 so the total HBM bytes equal one fp32
read of the tensor.

On-chip layout is "transposed" (feature-on-partitions):
    encT  [128(e), 16, L]   per batch  -> moving operand of the projection
    hidT  [128(a), 4, L]    per batch  -> moving operand of the att matvec
    att rows are gathered to [8, L], softmax'd batched, then alpha is
    PE-transposed and scattered into a block-diagonal [13*128, 8] stationary
    operand so the weighted sum runs as one accumulated matmul over the
    group's flattened (b, l) rows.
"""

import functools
import os

import ml_dtypes
import numpy as np

import concourse.bacc as bacc
import concourse.mybir as mybir
import concourse.tile as tile
from concourse.bass_utils import run_bass_kernel_spmd

N_CORES = 8
B, L, ENC, DEC, ATT = 256, 196, 2048, 512, 512
BC = B // N_CORES          # 32 batches per core
GROUP = 8                  # batches per awe-group
N_GROUPS = BC // GROUP     # 8
NPAIR = 2                  # batches processed per PE pass
# the weighted sum packs each batch's L=196 rows into a 224-row band
# (224 = 7*32) so a group is exactly 7 full 128-row chunks and every
# alpha^T fragment starts at a 32-aligned partition (DVE requirement)
PADL = 224
K_CHUNKS = GROUP * PADL // 128  # 7
OUT_W = ENC + L            # packed output row: awe | alpha


def _pieces(span):
    """Per group-local batch: (chunk, part, l0, n) pieces covering l in
    [0, span), split at 128-row chunk boundaries of the padded layout."""
    out = []
    for jj in range(GROUP):
        row0 = jj * PADL
        ps = []
        s = row0
        while s < row0 + span:
            e = min(row0 + span, (s // 128 + 1) * 128)
            ps.append((s // 128, s % 128, s - row0, e - s))
            s = e
        out.append(ps)
    return out


def _split_for_engine(pieces):
    """DVE/ACT partition windows: start 0 -> <=128, start 64 -> <=64,
    start 32/96 -> <=32. Split fragments to fit."""
    out = []
    for ps in pieces:
        cur = []
        for chunk, part, l0, n in ps:
            while n > 0:
                lim = 128 if part == 0 else 64 if part == 64 else 32
                take = min(n, lim)
                cur.append((chunk, part, l0, take))
                part += take
                l0 += take
                n -= take
        out.append(cur)
    return out


PIECES_VAL = _split_for_engine(_pieces(L))  # alpha fragments (32-aligned)
PIECES_PAD = _pieces(PADL)  # nat DMA coverage incl. pad rows
assert all(p % 32 == 0 for ps in PIECES_VAL for _, p, _, _ in ps)

F32 = mybir.dt.float32
BF16 = mybir.dt.bfloat16
FP8 = mybir.dt.float8e4
AX = mybir.AxisListType.X
AF = mybir.ActivationFunctionType

# fp8 (e4m3, DoubleRow) path for the big projection matmul: 2x PE rate.
# Whole-chain rel err ~1e-2 (vs 2.5e-3 bf16), still under the 2e-2 gate.
USE_FP8 = True
SWI = True                 # DoubleRowSwInterleave: host-interleaved weights
W_SCALE = 4096.0           # W_enc pre-scale so e4m3 sees normal-range values
LP = 208                   # L padded to a multiple of 16 for the DoubleRow AP

TRACE_ENV = "ATT_KERNEL_TRACE"
LAST_RESULTS = {}


def build_nc():
    nc = bacc.Bacc(
        "TRN2", target_bir_lowering=False, debug=False, num_devices=N_CORES
    )
    # enc_t host layout (bf16): [b][p][c][l] = enc[b, l, 128*c + p] so each
    # DMA is contiguous per partition (6272 B runs, not 392 B descriptors).
    # fp8 DoubleRow layout: [b][p][c8][r][l] = enc[b, l, 256*c8 + 2*p + r].
    if USE_FP8:
        # pair-interleaved: [pair][p][c8][r][jj][l] = enc[2*pair+jj, l, 256*c8+2*p+r]
        enc_t = nc.declare_dram_parameter(
            "enc_t", [BC // NPAIR, 128, ENC // 256, 2, NPAIR, LP], FP8, isOutput=False
        )
    else:
        enc_t = nc.declare_dram_parameter(
            "enc_t", [BC, 128, (ENC // 128) * L], BF16, isOutput=False
        )
    enc_nat = nc.declare_dram_parameter("enc_nat", [BC * L, ENC], BF16, isOutput=False)
    # packed small preloads, one DMA each:
    # f32 blob [128, 40]: b_enc [*,0:4] | b_dec [*,4:8] | ident rows [*,8:40]
    small_f32 = nc.declare_dram_parameter("small_f32", [128, 40], F32, isOutput=False)
    # bf16 blob [128, 132]: dec_h^T [*, 0:128] ([p][q][b] = dec[b, 128q+p]) |
    # w_att [*, 128:132]
    small_bf = nc.declare_dram_parameter("small_bf", [128, 132], BF16, isOutput=False)
    if USE_FP8:
        # m-major so the first m-slice (256 KB) lands fast at kernel start
        w_enc = nc.declare_dram_parameter(
            "w_enc", [ATT // 128, 128, ENC // 256, 256], FP8, isOutput=False
        )
    else:
        w_enc = nc.declare_dram_parameter("w_enc", [ENC, ATT], BF16, isOutput=False)
    w_dec = nc.declare_dram_parameter("w_dec", [DEC, ATT], BF16, isOutput=False)
    out_d = nc.declare_dram_parameter("out", [BC, OUT_W], F32, isOutput=True)

    with tile.TileContext(nc) as tc:
        with (
            tc.tile_pool(name="w", bufs=1) as wp,
            tc.tile_pool(name="encT", bufs=2) as encT_p,
            tc.tile_pool(name="nat", bufs=2) as nat_p,
            tc.tile_pool(name="hid", bufs=3) as hid_p,
            tc.tile_pool(name="bd", bufs=3) as bd_p,
            tc.tile_pool(name="sm", bufs=4) as sm_p,
            tc.tile_pool(name="awe", bufs=2) as awe_p,
            tc.tile_pool(name="psmm", bufs=3, space="PSUM") as ps_mm,
            tc.tile_pool(name="psatt", bufs=1, space="PSUM") as ps_att,
            tc.tile_pool(name="psawe", bufs=2, space="PSUM") as ps_awe,
            tc.tile_pool(name="pssm", bufs=1, space="PSUM") as ps_sm,
        ):
            # ---- constant / weight preloads: two packed small DMAs on the
            # gpsimd ring, w_enc m-slices first on sync so pair 0 starts fast
            sf = wp.tile([128, 40], F32)
            nc.gpsimd.dma_start(sf[:], small_f32.ap())
            sb = wp.tile([128, 132], BF16)
            nc.gpsimd.dma_start(sb[:], small_bf.ap())
            b_enc_sb = sf[:, 0:4]
            b_dec_sb = sf[:, 4:8]
            ident = sf[0:32, 8:40]
            dect_sb = sb[:, 0:128].rearrange("p (q b) -> p q b", b=BC)
            w_att_sb = sb[:, 128:132]
            if USE_FP8:
                w_enc_sb = wp.tile([128, ATT // 128, ENC // 256, 256], FP8)
                for m in range(ATT // 128):
                    nc.sync.dma_start(w_enc_sb[:, m], w_enc.ap()[m])
            else:
                w_enc_sb = wp.tile([128, ENC // 128, ATT], BF16)
                nc.sync.dma_start(
                    w_enc_sb[:], w_enc.ap().rearrange("(c p) a -> p c a", p=128)
                )
            w_dec_sb = wp.tile([128, DEC // 128, ATT], BF16)
            nc.sync.dma_start(
                w_dec_sb[:], w_dec.ap().rearrange("(c p) a -> p c a", p=128)
            )
            bias_sb = wp.tile([128, ATT // 128], F32)
            nc.vector.tensor_add(bias_sb[:], b_enc_sb, b_dec_sb)

            # dec_att2^T = W_dec^T @ dec_hidden^T + (b_enc + b_dec), f32: the
            # per-partition relu bias. Emitted late (inside pair 0, after its
            # first matmul block) so it does not gate the kernel start.
            dec2_sb = wp.tile([128, ATT // 128, BC], F32)

            def emit_dec_chain():
                for m in range(ATT // 128):
                    pm = ps_sm.tile([128, BC], F32, tag="sm", name=f"decmm{m}")
                    for k in range(DEC // 128):
                        nc.tensor.matmul(
                            pm[:],
                            w_dec_sb[:, k, m * 128 : (m + 1) * 128],
                            dect_sb[:, k, :],
                            start=(k == 0),
                            stop=(k == DEC // 128 - 1),
                        )
                    nc.vector.tensor_scalar_add(
                        dec2_sb[:, m, :], pm[:], bias_sb[:, m : m + 1]
                    )

            # nat rows for batch jj of group g: pad rows deliberately read the
            # following batch's rows (they are multiplied by bd zeros); the
            # very last batch clamps to re-read valid trailing rows instead
            def emit_nat_loads(g, jj, nat):
                b = g * GROUP + jj
                for chunk, part, l0, n in PIECES_PAD[jj]:
                    src0 = b * L + l0
                    if src0 + n <= BC * L:
                        nc.sync.dma_start(
                            nat[part : part + n, chunk, :],
                            enc_nat.ap()[src0 : src0 + n, :],
                        )
                    else:
                        nv = BC * L - src0
                        if nv > 0:
                            nc.sync.dma_start(
                                nat[part : part + nv, chunk, :],
                                enc_nat.ap()[src0 : src0 + nv, :],
                            )
                        npad = n - nv
                        nc.sync.dma_start(
                            nat[part + nv : part + n, chunk, :],
                            enc_nat.ap()[BC * L - npad : BC * L, :],
                        )

            # awe for group g, emitted one group late so the PE never waits
            # on the softmax -> alpha^T -> bd chain
            def do_awe(g, nat, bd):
                awe_sb = awe_p.tile([GROUP, ENC], F32, tag="awesb")
                for n in range(ENC // 512):
                    pw = ps_awe.tile([GROUP, 512], F32, tag="pw", name=f"pw_{g}_{n}")
                    for c in range(K_CHUNKS):
                        nc.tensor.matmul(
                            pw[:],
                            bd[:, c, :],
                            nat[:, c, n * 512 : (n + 1) * 512],
                            start=(c == 0),
                            stop=(c == K_CHUNKS - 1),
                        )
                    nc.vector.tensor_copy(awe_sb[:, n * 512 : (n + 1) * 512], pw[:])
                nc.sync.dma_start(
                    out_d.ap()[g * GROUP : (g + 1) * GROUP, 0:ENC], awe_sb[:]
                )

            # ---- main loop ------------------------------------------------
            pending = None
            dec_emitted = [False]

            def maybe_emit_dec():
                if not dec_emitted[0]:
                    dec_emitted[0] = True
                    emit_dec_chain()

            for g in range(N_GROUPS):
                # group's encoder rows, 128-padded per batch: chunk 2j holds
                # batch j's rows l=0:128, chunk 2j+1 rows l=128:196 (68 valid)
                # group's encoder rows for the weighted sum (consumed by
                # do_awe(g) one iteration later); sync ring so they never
                # delay the critical encT stream on gpsimd, interleaved into
                # the pair loop to smooth the queue
                nat = nat_p.tile([128, K_CHUNKS, ENC], BF16)

                bd = bd_p.tile([128, K_CHUNKS, GROUP], BF16)
                nc.vector.memset(bd[:], 0.0)

                for t in range(GROUP // NPAIR):
                    b0 = g * GROUP + NPAIR * t
                    hid = hid_p.tile([128, ATT // 128, NPAIR, L], BF16)
                    if USE_FP8:
                        encT = encT_p.tile(
                            [128, ENC // 256, 2, NPAIR, LP], FP8, tag="encT"
                        )
                        nc.gpsimd.dma_start(encT[:], enc_t.ap()[b0 // NPAIR])
                        # group 0: keep HBM clear for the critical first encT
                        # transfers; its nat loads are emitted after the pairs
                        if g > 0:
                            for jj in range(NPAIR * t, NPAIR * (t + 1)):
                                emit_nat_loads(g, jj, nat)
                        for m in range(ATT // 128):
                            pm = ps_mm.tile([128, NPAIR, LP], F32, tag="pm")
                            for c in range(ENC // 256):
                                if SWI:
                                    lw = w_enc_sb[:, m, c, :]
                                    pmode = mybir.MatmulPerfMode.DoubleRowSwInterleave
                                else:
                                    lw = w_enc_sb[:, m, c, :].rearrange(
                                        "p (r mm) -> p r mm", r=2
                                    )
                                    pmode = mybir.MatmulPerfMode.DoubleRow
                                nc.tensor.matmul(
                                    pm[:],
                                    lw,
                                    encT[:, c, :, :, :].rearrange(
                                        "p r j l -> p r (j l)"
                                    ),
                                    start=(c == 0),
                                    stop=(c == ENC // 256 - 1),
                                    perf_mode=pmode,
                                )
                            maybe_emit_dec()
                            for j in range(NPAIR):
                                nc.scalar.activation(
                                    hid[:, m, j, :],
                                    pm[:, j, 0:L],
                                    AF.Relu,
                                    bias=dec2_sb[:, m, b0 + j : b0 + j + 1],
                                    scale=1.0 / W_SCALE,
                                )
                    else:
                        encT = encT_p.tile(
                            [128, NPAIR, ENC // 128, L], BF16, tag="encT"
                        )
                        for j in range(NPAIR):
                            nc.gpsimd.dma_start(
                                encT[:, j, :, :],
                                enc_t.ap()[b0 + j].rearrange("p (c l) -> p c l", l=L),
                            )
                        for jj in range(NPAIR * t, NPAIR * (t + 1)):
                            b = g * GROUP + jj
                            nc.sync.dma_start(
                                nat[:, 2 * jj, :], enc_nat.ap()[b * L : b * L + L0, :]
                            )
                            nc.sync.dma_start(
                                nat[0:L1, 2 * jj + 1, :],
                                enc_nat.ap()[b * L + L0 : (b + 1) * L, :],
                            )
                        for m in range(ATT // 128):
                            pm = ps_mm.tile([128, NPAIR, L], F32, tag="pm")
                            for c in range(ENC // 128):
                                nc.tensor.matmul(
                                    pm[:],
                                    w_enc_sb[:, c, m * 128 : (m + 1) * 128],
                                    encT[:, :, c, :],
                                    start=(c == 0),
                                    stop=(c == ENC // 128 - 1),
                                )
                            maybe_emit_dec()
                            for j in range(NPAIR):
                                nc.scalar.activation(
                                    hid[:, m, j, :],
                                    pm[:, j, :],
                                    AF.Relu,
                                    bias=dec2_sb[:, m, b0 + j : b0 + j + 1],
                                    scale=1.0,
                                )
                    pa = ps_att.tile([1, NPAIR, L], F32)
                    for m in range(ATT // 128):
                        nc.tensor.matmul(
                            pa[:],
                            w_att_sb[:, m : m + 1],
                            hid[:, m, :, :],
                            start=(m == 0),
                            stop=(m == ATT // 128 - 1),
                        )

                    # softmax in row form (all ops live on partition 0)
                    negmax = sm_p.tile([1, NPAIR, 1], F32, tag="negmax")
                    nc.vector.reduce_max(negmax[:], pa[:], axis=AX, negate=True)
                    p_pr = sm_p.tile([1, NPAIR, L], F32, tag="ppr")
                    for j in range(NPAIR):
                        nc.scalar.activation(
                            p_pr[:, j, :], pa[:, j, :], AF.Exp,
                            bias=negmax[:, j, :], scale=1.0,
                        )
                    ssum = sm_p.tile([1, NPAIR, 1], F32, tag="ssum")
                    nc.vector.reduce_sum(ssum[:], p_pr[:], axis=AX)
                    rcp = sm_p.tile([1, NPAIR, 1], F32, tag="rcp")
                    nc.vector.reciprocal(rcp[:], ssum[:])
                    alpha_pr = sm_p.tile([1, NPAIR, L], F32, tag="alphapr")
                    for j in range(NPAIR):
                        nc.vector.tensor_scalar_mul(
                            alpha_pr[:, j, :], p_pr[:, j, :], rcp[:, j, :]
                        )
                    nc.sync.dma_start(
                        out_d.ap()[b0 : b0 + NPAIR, ENC : ENC + L], alpha_pr[:]
                    )
                    # alpha^T fragments into the block-diagonal stationary
                    # operand; every fragment starts 32-aligned by layout
                    for j in range(NPAIR):
                        jj = NPAIR * t + j
                        for idx, (chunk, part, l0, n) in enumerate(PIECES_VAL[jj]):
                            ta = ps_sm.tile(
                                [n, 1], F32, tag="at", name=f"at{g}_{jj}_{idx}"
                            )
                            nc.tensor.transpose(
                                ta[:], alpha_pr[:, j, l0 : l0 + n], ident[0:1, 0:1]
                            )
                            nc.vector.tensor_copy(
                                bd[part : part + n, chunk, jj : jj + 1], ta[:]
                            )

                if g == 0:
                    for jj in range(GROUP):
                        emit_nat_loads(g, jj, nat)
                if pending is not None:
                    do_awe(*pending)
                pending = (g, nat, bd)
            do_awe(*pending)

    nc.compile()
    return nc


@functools.cache
def _get_nc():
    return build_nc()


def make_in_maps(encoder_out, decoder_hidden, W_enc, b_enc, W_dec, b_dec, W_att):
    bf = ml_dtypes.bfloat16
    f8 = ml_dtypes.float8_e4m3
    enc = np.ascontiguousarray(np.asarray(encoder_out, dtype=np.float32))
    dec = np.ascontiguousarray(np.asarray(decoder_hidden, dtype=np.float32))
    if USE_FP8:
        # [m][p][c8][r][a'] = (W_SCALE * W)[256*c8 + 2*p + r, 128*m + a']
        w5 = (
            (np.asarray(W_enc, np.float32) * W_SCALE)
            .reshape(ENC // 256, 128, 2, ATT // 128, 128)
            .transpose(3, 1, 0, 2, 4)
        )
        if SWI:
            # hw flat layout: q = 2*(127 - a') + r  (A/B interleaved per
            # column, columns reversed — see CoreSim DoubleRowSwInterleave)
            w5 = w5[:, :, :, :, ::-1].transpose(0, 1, 2, 4, 3)
        w_enc_bf = np.ascontiguousarray(w5).astype(f8).reshape(
            ATT // 128, 128, ENC // 256, 256
        )
    else:
        w_enc_bf = np.ascontiguousarray(np.asarray(W_enc)).astype(bf)
    w_dec_bf = np.ascontiguousarray(np.asarray(W_dec)).astype(bf)
    w_att_bf = np.ascontiguousarray(np.asarray(W_att)).astype(bf)
    b_enc32 = np.ascontiguousarray(np.asarray(b_enc, dtype=np.float32))
    b_dec32 = np.ascontiguousarray(np.asarray(b_dec, dtype=np.float32))
    ident = np.eye(32, dtype=np.float32)
    in_maps = []
    for i in range(N_CORES):
        sh = enc[i * BC : (i + 1) * BC]
        sh_bf = sh.astype(bf)
        if USE_FP8:
            # [pair][p][c8][r][jj][l] = enc[2*pair+jj, l, 256*c8 + 2*p + r]
            et = np.zeros((BC // NPAIR, 128, ENC // 256, 2, NPAIR, LP), dtype=f8)
            et[..., :L] = (
                sh.astype(f8)
                .transpose(0, 2, 1)
                .reshape(BC // NPAIR, NPAIR, ENC // 256, 128, 2, L)
                .transpose(0, 3, 2, 4, 1, 5)
            )
            enc_t = et
        else:
            enc_t = np.ascontiguousarray(
                sh_bf.transpose(0, 2, 1)
                .reshape(BC, ENC // 128, 128, L)
                .transpose(0, 2, 1, 3)
            ).reshape(BC, 128, (ENC // 128) * L)
        in_maps.append(
            {
                "enc_t": enc_t,
                "enc_nat": sh_bf.reshape(BC * L, ENC),
                "small_f32": np.concatenate(
                    [
                        b_enc32.reshape(ATT // 128, 128).T,
                        b_dec32.reshape(ATT // 128, 128).T,
                        np.concatenate(
                            [ident, np.zeros((96, 32), np.float32)], axis=0
                        ),
                    ],
                    axis=1,
                ).astype(np.float32),
                "small_bf": np.concatenate(
                    [
                        np.ascontiguousarray(
                            dec[i * BC : (i + 1) * BC]
                            .T.reshape(DEC // 128, 128, BC)
                            .transpose(1, 0, 2)
                        ).reshape(128, 128),
                        w_att_bf.reshape(ATT // 128, 128).T,
                    ],
                    axis=1,
                ).astype(bf),
                "w_enc": w_enc_bf,
                "w_dec": w_dec_bf,
            }
        )
    return in_maps


def kernel(
    encoder_out, decoder_hidden, W_enc, b_enc, W_dec, b_dec, W_att, b_att=None
):
    nc = _get_nc()
    in_maps = make_in_maps(
        encoder_out, decoder_hidden, W_enc, b_enc, W_dec, b_dec, W_att
    )
    trace = bool(int(os.environ.get(TRACE_ENV, "0")))
    kw = {}
    if trace:
        kw["trace"] = True
    res = run_bass_kernel_spmd(nc, in_maps, core_ids=list(range(N_CORES)), **kw)
    LAST_RESULTS["exec_time_ns"] = res.exec_time_ns
    LAST_RESULTS["mean_exec_time_ns"] = res.mean_exec_time_ns
    full = np.concatenate([r["out"] for r in res.results], axis=0)
    awe = np.ascontiguousarray(full[:, :ENC])
    alpha = np.ascontiguousarray(full[:, ENC:])
    return awe, alpha
